# revision 18
# baseline (speedup 1.0000x reference)
"""DN4 retrieval-kNN layer as a Trainium2 Bass/Tile kernel (fp8 DoubleRow).

Reference computation (shapes hardcoded from the problem spec):
  query_feat  [t=4, wq=75, c=640, 10, 10]  -> q normalized over hw axis (per (wq, c))
  support_feat[t=4, ws=25, c=640, 10, 10]  -> s normalized over c axis (per (way, y))
  relation[t, wq, way, x, y] = sum_c qn[t, wq, x, c] * sn[t, way, c, y]   (x=100, y=500)
  score[t, wq, way] = sum_x sum(top3_y(relation))

Sharding: 8 cores = 4 episodes (t) x 2 query-halves. Core 2t handles queries
[0:38), core 2t+1 handles queries [37:75) (38 rows each; query 37 is computed
twice and deduplicated on the host). No cross-device communication.

Device kernel (per core), v2:
  - relation matmuls use N=512 tiles (support Y padded 500->512 with zeros):
    DoubleRow fp8 matmuls run 286ns at N=512 vs 375ns at N=500 on HW.
  - top-k is a split pipeline: per group, ROUTES assigns each way either
    'D' (DVE max8 direct on the fp32 PSUM tile, ~676ns) or 'A' (ACT copies
    PSUM->SBUF bf16 ~710ns, DVE tensor_max halves 512->256 ~140ns + max8 on
    256 ~233ns). The pairwise-max halving loses one of the top-3 only when
    two of them are exactly 256 apart (~0.6%/tile, negligible score impact).
  - query normalize: blocks either on DVE (mul+segmented reduce) or ACT
    (per-query Square with accum), chosen per block to balance engines.
"""

import sys
import numpy as np

sys.path.insert(0, "/opt/trn_rl_repo")

T, WQ, C, HW = 4, 75, 640, 100
WAY, SHOT = 5, 5
NS = WAY * SHOT          # 25 support images per episode
Y = SHOT * HW            # 500 support descriptors per way
YALL = WAY * Y           # 2500
QPC = 38                 # queries per core (overlapping halves of 75)
KC = C // 128            # 5 contraction chunks of 128
NCORES = 8
NK = 3                   # top-k
ROWS = QPC * HW          # 3800 flattened (query, x) relation rows per core
GROUPS = (ROWS + 127) // 128   # 30 row-groups of <=128
BK = 512                 # PSUM bank stride in fp32 elements
SQ = 16.0                # fp8 scale on normalized query
SS = 16.0                # fp8 scale on normalized support
SEG_DELAY = 3            # groups to delay the seg matmul behind max8
QBLOCKS = [8, 8, 8, 8, 6]  # query pipeline blocks (sum = QPC)
QB_ENG = "XXXXX"         # per-QBLOCK square engine: V=DVE mul+square,
                         # X=ACT Square + DVE reduce
S_SQ_ENG = "V"           # support squares: V=DVE, A=ACT
S_MUL_ENG = "PPPPP"      # engine per contraction chunk for the support mul
RBLK = 2048              # rows per weight block: LDWEIGHTS from tiles wider
                         # than 2048 runs ~60% slower (HW-measured), so the
                         # q8 weights are stored as two 2048-row blocks
NBLK = 2                 # ceil(ROWS / RBLK)
YPAD = 512               # s8 way stride; also the relation tile width (N=512)
ROUTES = "AADAD"         # per-way top-k route: A=ACT-copy+halve, D=direct max8

_PROGRAM = None


def _build_program(phases=3, loop_reps=0, loop_scope="main", variant=""):
    import concourse.tile as tile
    from concourse import bacc, mybir
    from contextlib import ExitStack, nullcontext

    fp32 = mybir.dt.float32
    bf16 = mybir.dt.bfloat16
    fp8 = mybir.dt.float8e4
    AF = mybir.ActivationFunctionType
    AX = mybir.AxisListType
    DR = mybir.MatmulPerfMode.DoubleRow

    nc = bacc.Bacc("TRN2", target_bir_lowering=False, debug=False)
    q_in = nc.declare_dram_parameter("q_in", [C, QPC, HW], bf16, isOutput=False)
    s_in = nc.declare_dram_parameter("s_in", [C, NS, HW], bf16, isOutput=False)
    seg_in = nc.declare_dram_parameter("seg_in", [128, GROUPS, QPC], bf16, isOutput=False)
    score_out = nc.declare_dram_parameter("score_out", [QPC, WAY], fp32, isOutput=True)

    with ExitStack() as ctx:
        tc = ctx.enter_context(tile.TileContext(nc))
        const = ctx.enter_context(tc.tile_pool(name="const", bufs=1))
        sbig = ctx.enter_context(tc.tile_pool(name="sbig", bufs=1))
        stage = ctx.enter_context(tc.tile_pool(name="stage", bufs=2))
        qscr = ctx.enter_context(tc.tile_pool(name="qscr", bufs=6))
        small = ctx.enter_context(tc.tile_pool(name="small", bufs=2))
        t8p = ctx.enter_context(tc.tile_pool(name="t8p", bufs=10))
        rbp = ctx.enter_context(tc.tile_pool(name="rbp", bufs=4))
        rbp = ctx.enter_context(tc.tile_pool(name="rbp", bufs=4))
        psp = ctx.enter_context(tc.tile_pool(name="psp", bufs=8, space="PSUM"))

        # Constants
        ones_k = const.tile([128, 1], bf16, name="ones_k")
        nc.vector.memset(ones_k[:], 1.0)
        ones_m = const.tile([1, 128], bf16, name="ones_m")
        nc.vector.memset(ones_m[:], 1.0)
        warm_rhs = const.tile([128, BK], fp8, name="warm_rhs")
        nc.gpsimd.memset(warm_rhs[:], 1.0)
        ones_k8 = const.tile([128, 1], fp8, name="ones_k8")
        nc.vector.memset(ones_k8[:], 1.0)

        # ------------- loads: q-head + s first (gate the normalize chains),
        # q-tail + seg stream behind them -------------
        QH = 8   # queries covered by the pre-main blocks
        sn = []
        qn = []
        for kc in range(KC):
            qnk = sbig.tile([128, QPC, HW], bf16, name=f"qn{kc}")
            qn.append(qnk)
            snk = sbig.tile([128, WAY, Y], bf16, name=f"sn{kc}")
            sn.append(snk)
        for kc in range(KC):
            nc.sync.dma_start(
                out=qn[kc][:, 0:QH], in_=q_in[kc * 128:(kc + 1) * 128, 0:QH]
            )
        for w in range(WAY):
            for kc in range(KC):
                nc.sync.dma_start(
                    out=sn[kc][:, w].rearrange("c (s x) -> c s x", x=HW),
                    in_=s_in[kc * 128:(kc + 1) * 128, w * SHOT:(w + 1) * SHOT],
                )
        for kc in range(KC):
            nc.sync.dma_start(
                out=qn[kc][:, QH:QPC], in_=q_in[kc * 128:(kc + 1) * 128, QH:QPC]
            )
        seg = sbig.tile([128, GROUPS, QPC], bf16, name="seg")
        nc.sync.dma_start(out=seg[:], in_=seg_in[:])

        # fp8 DoubleRow pair layouts. Rows are stored in NBLK blocks of RBLK
        # so every weight tile is exactly [128, 2, 2048] / [128, 2048] — the
        # LDWEIGHTS fast-path shape.
        q8p = [
            [sbig.tile([128, 2, RBLK], fp8, name=f"q8p{b}_{i}") for i in range(2)]
            for b in range(NBLK)
        ]
        q8l = [sbig.tile([128, RBLK], fp8, name=f"q8l{b}") for b in range(NBLK)]
        s8p = [sbig.tile([128, 2, WAY, YPAD], fp8, name=f"s8p{i}") for i in range(2)]
        s8l = sbig.tile([128, WAY, YPAD], fp8, name="s8l")
        # zero the Y..YPAD support pads so the relation pad columns are 0
        for i in range(2):
            for w in range(WAY):
                nc.vector.memset(s8p[i][:, :, w, Y:YPAD], 0.0)
        for w in range(WAY):
            nc.vector.memset(s8l[:, w, Y:YPAD], 0.0)

        # per-chunk persistent query-norm state
        ssqs = [sbig.tile([128, QPC], fp32, name=f"ssq{kc}") for kc in range(KC)]
        rqs = [sbig.tile([128, QPC], fp32, name=f"rq{kc}") for kc in range(KC)]

        body_cm = (
            tc.For_i(0, loop_reps, 1)
            if (loop_reps and loop_scope == "compute")
            else nullcontext()
        )
        with body_cm:
            # PE p-state warmup: ~2k cycles of dummy matmuls so the support
            # reduce and the first relation groups run at full clock
            warm_ps = psp.tile([1, BK], fp32, name="warm_ps", tag="rel")
            for i in range(12):
                nc.tensor.matmul(
                    warm_ps[:], lhsT=ones_k8[:], rhs=warm_rhs[:],
                    start=(i == 0), stop=(i == 11),
                )
            if phases >= 2 and "rawmm" in variant:
                for b in range(NBLK):
                    for i in range(2):
                        nc.gpsimd.memset(q8p[b][i][:], 0.25)
                    nc.gpsimd.memset(q8l[b][:], 0.25)
                for i in range(2):
                    nc.gpsimd.memset(s8p[i][:], 0.25)
                nc.gpsimd.memset(s8l[:], 0.25)
                piece_sched = {}
            if phases >= 2 and "rawmm" not in variant:
                QS, XS = RBLK // HW, RBLK % HW  # straddle query, split x

                def q_sq_piece(kc, q0, q1, eng="X"):
                    """sum-of-squares over hw for queries [q0,q1), one chunk"""
                    sqb = qscr.tile(
                        [128, 8 * HW], bf16, name="sqb", tag="sqb",
                    )[:, 0:(q1 - q0) * HW]
                    qflat = qn[kc][:, q0:q1].rearrange("c q x -> c (q x)")
                    if eng == "V":
                        nc.vector.tensor_mul(sqb, qflat, qflat)
                    else:
                        nc.scalar.activation(sqb, qflat, AF.Square)
                    nc.vector.reduce_sum(
                        ssqs[kc][:, q0:q1],
                        sqb.rearrange("c (q x) -> c q x", x=HW),
                        axis=AX.X,
                    )

                def q_emit_piece(kc, q0, q1, eng="X"):
                    """normalizer + fp8 emission for queries [q0,q1), one chunk"""
                    nc.vector.reciprocal(
                        rqs[kc][:, q0:q1], ssqs[kc][:, q0:q1]
                    )
                    nc.scalar.activation(
                        rqs[kc][:, q0:q1], rqs[kc][:, q0:q1],
                        AF.Sqrt, scale=SQ * SQ,
                    )

                    def blk_tile(b):
                        return (
                            q8p[b][kc // 2][:, kc % 2]
                            if kc < 4 else q8l[b][:]
                        )

                    def emit(dst, qa, qb, x0=0, x1=HW):
                        src = qn[kc][:, qa:qb, x0:x1] if (x0, x1) != (0, HW) \
                            else qn[kc][:, qa:qb]
                        nc.gpsimd.tensor_mul(
                            dst, src,
                            rqs[kc][:, qa:qb].unsqueeze(2)
                                .broadcast_to([128, qb - qa, x1 - x0]),
                        )

                    # block A: full queries [q0, min(q1, QS))
                    if q0 < min(q1, QS):
                        qa, qb = q0, min(q1, QS)
                        emit(
                            blk_tile(0)[:, 0:QS * HW]
                            .rearrange("c (q x) -> c q x", x=HW)[:, qa:qb],
                            qa, qb,
                        )
                    # straddling query QS: x [0, XS) in A, [XS, HW) in B
                    if q0 <= QS < q1:
                        emit(
                            blk_tile(0)[:, QS * HW:RBLK]
                            .unsqueeze(1), QS, QS + 1, 0, XS,
                        )
                        emit(
                            blk_tile(1)[:, 0:HW - XS]
                            .unsqueeze(1), QS, QS + 1, XS, HW,
                        )
                    # block B: full queries [max(q0, QS+1), q1)
                    if max(q0, QS + 1) < q1:
                        qa, qb = max(q0, QS + 1), q1
                        boff = (QS + 1) * HW - RBLK
                        emit(
                            blk_tile(1)[:, boff:boff + (QPC - QS - 1) * HW]
                            .rearrange("c (q x) -> c q x", x=HW)
                            [:, qa - QS - 1:qb - QS - 1],
                            qa, qb,
                        )

                # piece schedule: block 0 runs inside the support phase; later
                # blocks are chopped into 10 ~1us pieces placed into (group,
                # way) slots ending 2 groups before the block's rows are read
                piece_sched = {}
                qa = QBLOCKS[0]
                for b in range(1, len(QBLOCKS)):
                    q0b, q1b = qa, qa + QBLOCKS[b]
                    g_need = (q0b * HW) // 128
                    end_slot = (g_need - 2) * WAY
                    pieces = (
                        [(q_sq_piece, kc, q0b, q1b, QB_ENG[b]) for kc in range(KC)]
                        + [(q_emit_piece, kc, q0b, q1b, QB_ENG[b]) for kc in range(KC)]
                    )
                    s0 = end_slot - len(pieces) + 1
                    assert s0 > 0, (b, s0)
                    for j, pc in enumerate(pieces):
                        piece_sched.setdefault(s0 + j, []).append(pc)
                    qa = q1b

                # ---- support normalize, way-major so way 0 finishes first;
                # block-0 query squares interleave one chunk per way ----
                sqs = [
                    stage.tile([128, YALL], bf16, name=f"sq{kc}", tag="sq",
                               bufs=5)
                    for kc in range(KC)
                ]
                sqsum = stage.tile([128, YALL], bf16, name="sqsum", bufs=1)
                s_recip = small.tile([1, YALL], fp32, name="s_recip", bufs=1)
                s_rs = small.tile([1, YALL], bf16, name="s_rs", bufs=1)
                rs_sb = small.tile([128, WAY, Y], bf16, name="rs_sb", bufs=1)
                for yc in range(WAY):
                    ys = slice(yc * Y, (yc + 1) * Y)
                    for kc in range(KC):
                        if S_SQ_ENG == "V":
                            nc.vector.tensor_mul(
                                sqs[kc][:, ys], sn[kc][:, yc], sn[kc][:, yc])
                        else:
                            nc.scalar.activation(
                                sqs[kc][:, ys], sn[kc][:, yc], AF.Square)
                    nc.vector.tensor_add(
                        sqsum[:, ys], sqs[0][:, ys], sqs[1][:, ys])
                    for kc in range(2, KC):
                        nc.vector.tensor_add(
                            sqsum[:, ys], sqsum[:, ys], sqs[kc][:, ys])
                    ss = psp.tile([1, BK], fp32, name=f"ss{yc}", tag="rel")
                    nc.tensor.matmul(
                        ss[:, 0:Y], lhsT=ones_k[:], rhs=sqsum[:, ys],
                        start=True, stop=True,
                    )
                    nc.vector.reciprocal(s_recip[:, ys], ss[:, 0:Y])
                    nc.scalar.activation(
                        s_rs[:, ys], s_recip[:, ys], AF.Sqrt, scale=SS * SS,
                    )
                    rb = psp.tile([128, BK], fp32, name=f"rs_bc{yc}", tag="rel")
                    nc.tensor.matmul(
                        rb[:, 0:Y], lhsT=ones_m[:], rhs=s_rs[:, ys],
                        start=True, stop=True,
                    )
                    nc.scalar.copy(rs_sb[:, yc], rb[:, 0:Y])
                    if yc in (1, 3):
                        # keep the PE's HAM activity window alive through the
                        # support-finalize idle gaps (redistributed warmup)
                        wp2 = psp.tile([1, BK], fp32, name=f"warm2_{yc}", tag="rel")
                        nc.tensor.matmul(
                            wp2[:], lhsT=ones_k[:], rhs=warm_rhs[:],
                            start=True, stop=True,
                        )
                    for kc in range(KC):
                        s8_dst = (
                            s8p[kc // 2][:, kc % 2, yc, 0:Y]
                            if kc < 4 else s8l[:, yc, 0:Y]
                        )
                        eng = nc.vector if S_MUL_ENG[kc] == "V" else nc.gpsimd
                        eng.tensor_mul(s8_dst, sn[kc][:, yc], rs_sb[:, yc])
                    # one chunk of the first query block per way
                    q_sq_piece(yc, 0, QBLOCKS[0], QB_ENG[0])
                for kc in range(KC):
                    q_emit_piece(kc, 0, QBLOCKS[0], QB_ENG[0])

            if phases <= 2:
                score_sb = small.tile([QPC, WAY], fp32, name="score_sb")
                nc.vector.tensor_copy(score_sb[:], s8l[0:QPC, 0, 0:WAY])
                nc.sync.dma_start(out=score_out[:], in_=score_sb[:])

            # ------------- main loop: fp8 relation matmuls + top-8 -------------
            if phases >= 3:
                score_ps = psp.tile([QPC, WAY * 8], fp32, name="score_ps", tag="rel")
                loop_cm = (
                    tc.For_i(0, loop_reps, 1)
                    if (loop_reps and loop_scope == "main")
                    else nullcontext()
                )
                with loop_cm:
                    t8qs = [None] * GROUPS
                    rel_dummy = None
                    if "nomm" in variant:
                        rel_dummy = psp.tile([128, YPAD], fp32, name="rel_d", tag="rel")
                        nc.vector.memset(rel_dummy[:], 0.5)

                    def seg_mm(g):
                        m = min(128, ROWS - g * 128)
                        nc.tensor.matmul(
                            score_ps[:],
                            lhsT=seg[0:m, g],
                            rhs=t8qs[g][0:m],
                            start=(g == 0),
                            stop=(g == GROUPS - 1),
                        )

                    for g in range(GROUPS):
                        m = min(128, ROWS - g * 128)
                        t8q = t8p.tile([128, WAY * 8], bf16, name="t8q")
                        t8qs[g] = t8q
                        for w in range(WAY):
                            rel = (
                                rel_dummy if variant == "nomm"
                                else psp.tile([128, YPAD], fp32, name="rel", tag="rel")
                            )
                            if "nomm" not in variant:
                                b = (g * 128) // RBLK
                                off = g * 128 - b * RBLK
                                nc.tensor.matmul(
                                    rel[0:m],
                                    lhsT=q8p[b][0][:, :, off:off + m],
                                    rhs=s8p[0][:, :, w],
                                    start=True, stop=False, perf_mode=DR,
                                )
                                nc.tensor.matmul(
                                    rel[0:m],
                                    lhsT=q8p[b][1][:, :, off:off + m],
                                    rhs=s8p[1][:, :, w],
                                    start=False, stop=False, perf_mode=DR,
                                )
                                nc.tensor.matmul(
                                    rel[0:m],
                                    lhsT=q8l[b][:, off:off + m],
                                    rhs=s8l[:, w],
                                    start=False, stop=True,
                                )
                            if "nomax" not in variant:
                                if ROUTES[w] == "A":
                                    relb = rbp.tile(
                                        [128, YPAD], bf16, name="relb", tag="relb",
                                    )
                                    nc.scalar.copy(relb[0:m], rel[0:m])
                                    red = rbp.tile(
                                        [128, 256], bf16, name="red", tag="red",
                                    )
                                    nc.vector.tensor_max(
                                        red[0:m], relb[0:m, 0:256],
                                        relb[0:m, 256:512],
                                    )
                                    nc.vector.max(
                                        t8q[0:m, w * 8:(w + 1) * 8], red[0:m]
                                    )
                                else:
                                    nc.vector.max(
                                        t8q[0:m, w * 8:(w + 1) * 8], rel[0:m]
                                    )
                            if phases >= 2 and "rawmm" not in variant:
                                for fn, kc_, q0_, q1_, eng_ in piece_sched.get(
                                        g * WAY + w, ()):
                                    fn(kc_, q0_, q1_, eng_)
                        if "nomax" not in variant:
                            if g >= SEG_DELAY:
                                seg_mm(g - SEG_DELAY)
                    if "nomax" not in variant:
                        for g in range(GROUPS - SEG_DELAY, GROUPS):
                            seg_mm(g)
                score_sb = small.tile([QPC, WAY], fp32, name="score_sb")
                if "nomax" in variant:
                    nc.vector.memset(score_sb[:], 0.0)
                else:
                    nc.vector.reduce_sum(
                        score_sb[:],
                        score_ps[:].rearrange("q (w k) -> q w k", k=8)[:, :, 0:NK],
                        axis=AX.X,
                    )
        if phases >= 3:
            nc.sync.dma_start(out=score_out[:], in_=score_sb[:])

    nc.compile()
    return nc


def _get_program():
    global _PROGRAM
    if _PROGRAM is None:
        _PROGRAM = _build_program()
    return _PROGRAM


def _seg_matrix():
    import ml_dtypes
    seg = np.zeros((128, GROUPS, QPC), dtype=np.float32)
    for r in range(ROWS):
        seg[r % 128, r // 128, r // HW] = 1.0 / (SQ * SS)
    return seg.astype(ml_dtypes.bfloat16)


def _make_in_maps(qf, sf):
    import ml_dtypes
    bf = ml_dtypes.bfloat16
    seg = _seg_matrix()
    in_maps = []
    for core in range(NCORES):
        t = core // 2
        q0 = 0 if core % 2 == 0 else WQ - QPC  # 0 or 37
        in_maps.append({
            "q_in": np.ascontiguousarray(
                qf[t, q0:q0 + QPC].transpose(1, 0, 2).astype(bf)),
            "s_in": np.ascontiguousarray(
                sf[t].transpose(1, 0, 2).astype(bf)),
            "seg_in": seg,
        })
    return in_maps


def kernel(query_feat, support_feat, way_num, shot_num, query_num, **_):
    from concourse.bass_utils import run_bass_kernel_spmd

    qf = np.asarray(query_feat, dtype=np.float32).reshape(T, WQ, C, HW)
    sf = np.asarray(support_feat, dtype=np.float32).reshape(T, NS, C, HW)
    assert int(way_num) == WAY and int(shot_num) == SHOT

    in_maps = _make_in_maps(qf, sf)
    res = run_bass_kernel_spmd(_get_program(), in_maps, list(range(NCORES))).results

    out = np.empty((T, WQ, WAY), dtype=np.float32)
    for t in range(T):
        lo = res[2 * t]["score_out"]
        hi = res[2 * t + 1]["score_out"]
        out[t, :QPC] = lo
        out[t, QPC:] = hi[QPC - (WQ - QPC):]  # drop the overlapping query row
    return out


# revision 19
# speedup vs baseline: 1.1887x; 1.1887x over previous
"""DN4 retrieval-kNN layer as a Trainium2 Bass/Tile kernel (fp8 DoubleRow).

Reference computation (shapes hardcoded from the problem spec):
  query_feat  [t=4, wq=75, c=640, 10, 10]  -> q normalized over hw axis (per (wq, c))
  support_feat[t=4, ws=25, c=640, 10, 10]  -> s normalized over c axis (per (way, y))
  relation[t, wq, way, x, y] = sum_c qn[t, wq, x, c] * sn[t, way, c, y]   (x=100, y=500)
  score[t, wq, way] = sum_x sum(top3_y(relation))

Sharding: 8 cores = 4 episodes (t) x 2 query-halves. Core 2t handles queries
[0:38), core 2t+1 handles queries [37:75) (38 rows each; query 37 is computed
twice and deduplicated on the host). No cross-device communication.

Device kernel (per core), v2:
  - relation matmuls use N=512 tiles (support Y padded 500->512 with zeros):
    DoubleRow fp8 matmuls run 286ns at N=512 vs 375ns at N=500 on HW.
  - top-k is a split pipeline: per group, ROUTES assigns each way either
    'D' (DVE max8 direct on the fp32 PSUM tile, ~676ns) or 'A' (ACT copies
    PSUM->SBUF bf16 ~710ns, DVE tensor_max halves 512->256 ~140ns + max8 on
    256 ~233ns). The pairwise-max halving loses one of the top-3 only when
    two of them are exactly 256 apart (~0.6%/tile, negligible score impact).
  - query normalize: blocks either on DVE (mul+segmented reduce) or ACT
    (per-query Square with accum), chosen per block to balance engines.
"""

import sys
import numpy as np

sys.path.insert(0, "/opt/trn_rl_repo")

T, WQ, C, HW = 4, 75, 640, 100
WAY, SHOT = 5, 5
NS = WAY * SHOT          # 25 support images per episode
Y = SHOT * HW            # 500 support descriptors per way
YALL = WAY * Y           # 2500
QPC = 38                 # queries per core (overlapping halves of 75)
KC = C // 128            # 5 contraction chunks of 128
NCORES = 8
NK = 3                   # top-k
ROWS = QPC * HW          # 3800 flattened (query, x) relation rows per core
GROUPS = (ROWS + 127) // 128   # 30 row-groups of <=128
BK = 512                 # PSUM bank stride in fp32 elements
SQ = 16.0                # fp8 scale on normalized query
SS = 16.0                # fp8 scale on normalized support
SEG_DELAY = 3            # groups to delay the seg matmul behind max8
QBLOCKS = [8, 8, 8, 8, 6]  # query pipeline blocks (sum = QPC)
QB_ENG = "XXXXX"         # per-QBLOCK square engine: V=DVE mul+square,
                         # X=ACT Square + DVE reduce
S_SQ_ENG = "V"           # support squares: V=DVE, A=ACT
S_MUL_ENG = "PPPPP"      # engine per contraction chunk for the support mul
RBLK = 2048              # rows per weight block: LDWEIGHTS from tiles wider
                         # than 2048 runs ~60% slower (HW-measured), so the
                         # q8 weights are stored as two 2048-row blocks
NBLK = 2                 # ceil(ROWS / RBLK)
YPAD = 512               # s8 way stride; also the relation tile width (N=512)
ROUTES = "AADAD"         # per-way top-k route: A=ACT-copy+halve, D=direct max8

_PROGRAM = None


def _build_program(phases=3, loop_reps=0, loop_scope="main", variant=""):
    import concourse.tile as tile
    from concourse import bacc, mybir
    from contextlib import ExitStack, nullcontext

    fp32 = mybir.dt.float32
    bf16 = mybir.dt.bfloat16
    fp8 = mybir.dt.float8e4
    AF = mybir.ActivationFunctionType
    AX = mybir.AxisListType
    DR = mybir.MatmulPerfMode.DoubleRow

    nc = bacc.Bacc("TRN2", target_bir_lowering=False, debug=False)
    q_in = nc.declare_dram_parameter("q_in", [C, QPC, HW], bf16, isOutput=False)
    s_in = nc.declare_dram_parameter("s_in", [C, NS, HW], bf16, isOutput=False)
    seg_in = nc.declare_dram_parameter("seg_in", [128, GROUPS, QPC], bf16, isOutput=False)
    score_out = nc.declare_dram_parameter("score_out", [QPC, WAY], fp32, isOutput=True)

    with ExitStack() as ctx:
        tc = ctx.enter_context(tile.TileContext(nc))
        const = ctx.enter_context(tc.tile_pool(name="const", bufs=1))
        sbig = ctx.enter_context(tc.tile_pool(name="sbig", bufs=1))
        stage = ctx.enter_context(tc.tile_pool(name="stage", bufs=2))
        qscr = ctx.enter_context(tc.tile_pool(name="qscr", bufs=6))
        small = ctx.enter_context(tc.tile_pool(name="small", bufs=2))
        t8p = ctx.enter_context(tc.tile_pool(name="t8p", bufs=10))
        rbp = ctx.enter_context(tc.tile_pool(name="rbp", bufs=4))
        rbp = ctx.enter_context(tc.tile_pool(name="rbp", bufs=4))
        psp = ctx.enter_context(tc.tile_pool(name="psp", bufs=8, space="PSUM"))

        # Constants
        ones_k = const.tile([128, 1], bf16, name="ones_k")
        nc.vector.memset(ones_k[:], 1.0)
        ones_m = const.tile([1, 128], bf16, name="ones_m")
        nc.vector.memset(ones_m[:], 1.0)
        warm_rhs = const.tile([128, BK], fp8, name="warm_rhs")
        nc.gpsimd.memset(warm_rhs[:], 1.0)
        ones_k8 = const.tile([128, 1], fp8, name="ones_k8")
        nc.vector.memset(ones_k8[:], 1.0)

        # ------------- loads: q-head + s first (gate the normalize chains),
        # q-tail + seg stream behind them -------------
        QH = 8   # queries covered by the pre-main blocks
        sn = []
        qn = []
        for kc in range(KC):
            qnk = sbig.tile([128, QPC, HW], bf16, name=f"qn{kc}")
            qn.append(qnk)
            snk = sbig.tile([128, WAY, Y], bf16, name=f"sn{kc}")
            sn.append(snk)
        for kc in range(KC):
            nc.sync.dma_start(
                out=qn[kc][:, 0:QH], in_=q_in[kc * 128:(kc + 1) * 128, 0:QH]
            )
        for w in range(WAY):
            for kc in range(KC):
                nc.sync.dma_start(
                    out=sn[kc][:, w].rearrange("c (s x) -> c s x", x=HW),
                    in_=s_in[kc * 128:(kc + 1) * 128, w * SHOT:(w + 1) * SHOT],
                )
        for kc in range(KC):
            nc.sync.dma_start(
                out=qn[kc][:, QH:QPC], in_=q_in[kc * 128:(kc + 1) * 128, QH:QPC]
            )
        seg = sbig.tile([128, GROUPS, QPC], bf16, name="seg")
        nc.sync.dma_start(out=seg[:], in_=seg_in[:])

        # fp8 DoubleRow pair layouts. Rows are stored in NBLK blocks of RBLK
        # so every weight tile is exactly [128, 2, 2048] / [128, 2048] — the
        # LDWEIGHTS fast-path shape.
        q8p = [
            [sbig.tile([128, 2, RBLK], fp8, name=f"q8p{b}_{i}") for i in range(2)]
            for b in range(NBLK)
        ]
        q8l = [sbig.tile([128, RBLK], fp8, name=f"q8l{b}") for b in range(NBLK)]
        s8p = [sbig.tile([128, 2, WAY, YPAD], fp8, name=f"s8p{i}") for i in range(2)]
        s8l = sbig.tile([128, WAY, YPAD], fp8, name="s8l")
        # zero the Y..YPAD support pads so the relation pad columns are 0
        for i in range(2):
            for w in range(WAY):
                nc.vector.memset(s8p[i][:, :, w, Y:YPAD], 0.0)
        for w in range(WAY):
            nc.vector.memset(s8l[:, w, Y:YPAD], 0.0)

        # per-chunk persistent query-norm state
        ssqs = [sbig.tile([128, QPC], fp32, name=f"ssq{kc}") for kc in range(KC)]
        rqs = [sbig.tile([128, QPC], fp32, name=f"rq{kc}") for kc in range(KC)]

        body_cm = (
            tc.For_i(0, loop_reps, 1)
            if (loop_reps and loop_scope == "compute")
            else nullcontext()
        )
        with body_cm:
            # PE p-state warmup: ~2k cycles of dummy matmuls so the support
            # reduce and the first relation groups run at full clock
            warm_ps = psp.tile([1, BK], fp32, name="warm_ps", tag="rel")
            for i in range(12):
                nc.tensor.matmul(
                    warm_ps[:], lhsT=ones_k8[:], rhs=warm_rhs[:],
                    start=(i == 0), stop=(i == 11),
                )
            if phases >= 2 and "rawmm" in variant:
                for b in range(NBLK):
                    for i in range(2):
                        nc.gpsimd.memset(q8p[b][i][:], 0.25)
                    nc.gpsimd.memset(q8l[b][:], 0.25)
                for i in range(2):
                    nc.gpsimd.memset(s8p[i][:], 0.25)
                nc.gpsimd.memset(s8l[:], 0.25)
                piece_sched = {}
            if phases >= 2 and "rawmm" not in variant:
                QS, XS = RBLK // HW, RBLK % HW  # straddle query, split x

                def q_sq_piece(kc, q0, q1, eng="X"):
                    """sum-of-squares over hw for queries [q0,q1), one chunk"""
                    sqb = qscr.tile(
                        [128, 8 * HW], bf16, name="sqb", tag="sqb",
                    )[:, 0:(q1 - q0) * HW]
                    qflat = qn[kc][:, q0:q1].rearrange("c q x -> c (q x)")
                    if eng == "V":
                        nc.vector.tensor_mul(sqb, qflat, qflat)
                    else:
                        nc.scalar.activation(sqb, qflat, AF.Square)
                    nc.vector.reduce_sum(
                        ssqs[kc][:, q0:q1],
                        sqb.rearrange("c (q x) -> c q x", x=HW),
                        axis=AX.X,
                    )

                def q_emit_piece(kc, q0, q1, eng="X"):
                    """normalizer + fp8 emission for queries [q0,q1), one chunk"""
                    nc.vector.reciprocal(
                        rqs[kc][:, q0:q1], ssqs[kc][:, q0:q1]
                    )
                    nc.scalar.activation(
                        rqs[kc][:, q0:q1], rqs[kc][:, q0:q1],
                        AF.Sqrt, scale=SQ * SQ,
                    )

                    def blk_tile(b):
                        return (
                            q8p[b][kc // 2][:, kc % 2]
                            if kc < 4 else q8l[b][:]
                        )

                    def emit(dst, qa, qb, x0=0, x1=HW):
                        src = qn[kc][:, qa:qb, x0:x1] if (x0, x1) != (0, HW) \
                            else qn[kc][:, qa:qb]
                        nc.gpsimd.tensor_mul(
                            dst, src,
                            rqs[kc][:, qa:qb].unsqueeze(2)
                                .broadcast_to([128, qb - qa, x1 - x0]),
                        )

                    # block A: full queries [q0, min(q1, QS))
                    if q0 < min(q1, QS):
                        qa, qb = q0, min(q1, QS)
                        emit(
                            blk_tile(0)[:, 0:QS * HW]
                            .rearrange("c (q x) -> c q x", x=HW)[:, qa:qb],
                            qa, qb,
                        )
                    # straddling query QS: x [0, XS) in A, [XS, HW) in B
                    if q0 <= QS < q1:
                        emit(
                            blk_tile(0)[:, QS * HW:RBLK]
                            .unsqueeze(1), QS, QS + 1, 0, XS,
                        )
                        emit(
                            blk_tile(1)[:, 0:HW - XS]
                            .unsqueeze(1), QS, QS + 1, XS, HW,
                        )
                    # block B: full queries [max(q0, QS+1), q1)
                    if max(q0, QS + 1) < q1:
                        qa, qb = max(q0, QS + 1), q1
                        boff = (QS + 1) * HW - RBLK
                        emit(
                            blk_tile(1)[:, boff:boff + (QPC - QS - 1) * HW]
                            .rearrange("c (q x) -> c q x", x=HW)
                            [:, qa - QS - 1:qb - QS - 1],
                            qa, qb,
                        )

                # piece schedule: block 0 runs inside the support phase; later
                # blocks are chopped into 10 ~1us pieces placed into (group,
                # way) slots ending 2 groups before the block's rows are read
                piece_sched = {}
                qa = QBLOCKS[0]
                for b in range(1, len(QBLOCKS)):
                    q0b, q1b = qa, qa + QBLOCKS[b]
                    g_need = (q0b * HW) // 128
                    end_slot = (g_need - 2) * WAY
                    pieces = (
                        [(q_sq_piece, kc, q0b, q1b, QB_ENG[b]) for kc in range(KC)]
                        + [(q_emit_piece, kc, q0b, q1b, QB_ENG[b]) for kc in range(KC)]
                    )
                    s0 = end_slot - len(pieces) + 1
                    assert s0 > 0, (b, s0)
                    for j, pc in enumerate(pieces):
                        piece_sched.setdefault(s0 + j, []).append(pc)
                    qa = q1b

                # ---- support normalize, way-major so way 0 finishes first;
                # block-0 query squares interleave one chunk per way ----
                sqs = [
                    stage.tile([128, YALL], bf16, name=f"sq{kc}", tag="sq",
                               bufs=5)
                    for kc in range(KC)
                ]
                sqsum = stage.tile([128, YALL], bf16, name="sqsum", bufs=1)
                s_recip = small.tile([1, YALL], fp32, name="s_recip", bufs=1)
                s_rs = small.tile([1, YALL], bf16, name="s_rs", bufs=1)
                rs_sb = small.tile([128, WAY, Y], bf16, name="rs_sb", bufs=1)
                for yc in range(WAY):
                    ys = slice(yc * Y, (yc + 1) * Y)
                    for kc in range(KC):
                        if S_SQ_ENG == "V":
                            nc.vector.tensor_mul(
                                sqs[kc][:, ys], sn[kc][:, yc], sn[kc][:, yc])
                        else:
                            nc.scalar.activation(
                                sqs[kc][:, ys], sn[kc][:, yc], AF.Square)
                    nc.vector.tensor_add(
                        sqsum[:, ys], sqs[0][:, ys], sqs[1][:, ys])
                    for kc in range(2, KC):
                        nc.vector.tensor_add(
                            sqsum[:, ys], sqsum[:, ys], sqs[kc][:, ys])
                    ss = psp.tile([1, BK], fp32, name=f"ss{yc}", tag="rel")
                    nc.tensor.matmul(
                        ss[:, 0:Y], lhsT=ones_k[:], rhs=sqsum[:, ys],
                        start=True, stop=True,
                    )
                    nc.vector.reciprocal(s_recip[:, ys], ss[:, 0:Y])
                    nc.scalar.activation(
                        s_rs[:, ys], s_recip[:, ys], AF.Sqrt, scale=SS * SS,
                    )
                    rb = psp.tile([128, BK], fp32, name=f"rs_bc{yc}", tag="rel")
                    nc.tensor.matmul(
                        rb[:, 0:Y], lhsT=ones_m[:], rhs=s_rs[:, ys],
                        start=True, stop=True,
                    )
                    nc.scalar.copy(rs_sb[:, yc], rb[:, 0:Y])
                    for kc in range(KC):
                        s8_dst = (
                            s8p[kc // 2][:, kc % 2, yc, 0:Y]
                            if kc < 4 else s8l[:, yc, 0:Y]
                        )
                        eng = nc.vector if S_MUL_ENG[kc] == "V" else nc.gpsimd
                        eng.tensor_mul(s8_dst, sn[kc][:, yc], rs_sb[:, yc])
                    # one chunk of the first query block per way
                    q_sq_piece(yc, 0, QBLOCKS[0], QB_ENG[0])
                for kc in range(KC):
                    q_emit_piece(kc, 0, QBLOCKS[0], QB_ENG[0])

            if phases <= 2:
                score_sb = small.tile([QPC, WAY], fp32, name="score_sb")
                nc.vector.tensor_copy(score_sb[:], s8l[0:QPC, 0, 0:WAY])
                nc.sync.dma_start(out=score_out[:], in_=score_sb[:])

            # ------------- main loop: fp8 relation matmuls + top-8 -------------
            if phases >= 3:
                score_ps = psp.tile([QPC, WAY * 8], fp32, name="score_ps", tag="rel")
                loop_cm = (
                    tc.For_i(0, loop_reps, 1)
                    if (loop_reps and loop_scope == "main")
                    else nullcontext()
                )
                with loop_cm:
                    t8qs = [None] * GROUPS
                    rel_dummy = None
                    if "nomm" in variant:
                        rel_dummy = psp.tile([128, YPAD], fp32, name="rel_d", tag="rel")
                        nc.vector.memset(rel_dummy[:], 0.5)

                    def seg_mm(g):
                        m = min(128, ROWS - g * 128)
                        nc.tensor.matmul(
                            score_ps[:],
                            lhsT=seg[0:m, g],
                            rhs=t8qs[g][0:m],
                            start=(g == 0),
                            stop=(g == GROUPS - 1),
                        )

                    for g in range(GROUPS):
                        m = min(128, ROWS - g * 128)
                        t8q = t8p.tile([128, WAY * 8], bf16, name="t8q")
                        t8qs[g] = t8q
                        for w in range(WAY):
                            rel = (
                                rel_dummy if variant == "nomm"
                                else psp.tile([128, YPAD], fp32, name="rel", tag="rel")
                            )
                            if "nomm" not in variant:
                                b = (g * 128) // RBLK
                                off = g * 128 - b * RBLK
                                nc.tensor.matmul(
                                    rel[0:m],
                                    lhsT=q8p[b][0][:, :, off:off + m],
                                    rhs=s8p[0][:, :, w],
                                    start=True, stop=False, perf_mode=DR,
                                )
                                nc.tensor.matmul(
                                    rel[0:m],
                                    lhsT=q8p[b][1][:, :, off:off + m],
                                    rhs=s8p[1][:, :, w],
                                    start=False, stop=False, perf_mode=DR,
                                )
                                nc.tensor.matmul(
                                    rel[0:m],
                                    lhsT=q8l[b][:, off:off + m],
                                    rhs=s8l[:, w],
                                    start=False, stop=True,
                                )
                            if "nomax" not in variant:
                                if ROUTES[w] == "A":
                                    relb = rbp.tile(
                                        [128, YPAD], bf16, name="relb", tag="relb",
                                    )
                                    nc.scalar.copy(relb[0:m], rel[0:m])
                                    red = rbp.tile(
                                        [128, 256], bf16, name="red", tag="red",
                                    )
                                    nc.vector.tensor_max(
                                        red[0:m], relb[0:m, 0:256],
                                        relb[0:m, 256:512],
                                    )
                                    nc.vector.max(
                                        t8q[0:m, w * 8:(w + 1) * 8], red[0:m]
                                    )
                                else:
                                    nc.vector.max(
                                        t8q[0:m, w * 8:(w + 1) * 8], rel[0:m]
                                    )
                            if phases >= 2 and "rawmm" not in variant:
                                for fn, kc_, q0_, q1_, eng_ in piece_sched.get(
                                        g * WAY + w, ()):
                                    fn(kc_, q0_, q1_, eng_)
                        if "nomax" not in variant:
                            if g >= SEG_DELAY:
                                seg_mm(g - SEG_DELAY)
                    if "nomax" not in variant:
                        for g in range(GROUPS - SEG_DELAY, GROUPS):
                            seg_mm(g)
                score_sb = small.tile([QPC, WAY], fp32, name="score_sb")
                if "nomax" in variant:
                    nc.vector.memset(score_sb[:], 0.0)
                else:
                    nc.vector.reduce_sum(
                        score_sb[:],
                        score_ps[:].rearrange("q (w k) -> q w k", k=8)[:, :, 0:NK],
                        axis=AX.X,
                    )
        if phases >= 3:
            nc.sync.dma_start(out=score_out[:], in_=score_sb[:])

    nc.compile()
    return nc


def _get_program():
    global _PROGRAM
    if _PROGRAM is None:
        _PROGRAM = _build_program()
    return _PROGRAM


def _seg_matrix():
    import ml_dtypes
    seg = np.zeros((128, GROUPS, QPC), dtype=np.float32)
    for r in range(ROWS):
        seg[r % 128, r // 128, r // HW] = 1.0 / (SQ * SS)
    return seg.astype(ml_dtypes.bfloat16)


def _make_in_maps(qf, sf):
    import ml_dtypes
    bf = ml_dtypes.bfloat16
    seg = _seg_matrix()
    in_maps = []
    for core in range(NCORES):
        t = core // 2
        q0 = 0 if core % 2 == 0 else WQ - QPC  # 0 or 37
        in_maps.append({
            "q_in": np.ascontiguousarray(
                qf[t, q0:q0 + QPC].transpose(1, 0, 2).astype(bf)),
            "s_in": np.ascontiguousarray(
                sf[t].transpose(1, 0, 2).astype(bf)),
            "seg_in": seg,
        })
    return in_maps


def kernel(query_feat, support_feat, way_num, shot_num, query_num, **_):
    from concourse.bass_utils import run_bass_kernel_spmd

    qf = np.asarray(query_feat, dtype=np.float32).reshape(T, WQ, C, HW)
    sf = np.asarray(support_feat, dtype=np.float32).reshape(T, NS, C, HW)
    assert int(way_num) == WAY and int(shot_num) == SHOT

    in_maps = _make_in_maps(qf, sf)
    res = run_bass_kernel_spmd(_get_program(), in_maps, list(range(NCORES))).results

    out = np.empty((T, WQ, WAY), dtype=np.float32)
    for t in range(T):
        lo = res[2 * t]["score_out"]
        hi = res[2 * t + 1]["score_out"]
        out[t, :QPC] = lo
        out[t, QPC:] = hi[QPC - (WQ - QPC):]  # drop the overlapping query row
    return out
